# revision 1
# baseline (speedup 1.0000x reference)
"""Bass/Tile kernel for nn_Colorizer (sparse deformable attention colorizer).

Sharding: spatial row-sharding across 8 cores; core i owns output rows
[7i, 7i+7). All refs computed on every core for its rows; the final joint
softmax is additive across refs so each core normalizes locally.

Per-core pipeline:
  A. CV volume (search ref): banded PE matmuls -> CV[pixel, row, dx(105)]
     per pair-group -> SBUF -> DRAM.
  B. Phase-1 gather (static idx): stride-3 rows of CV -> cc0 -> exp ->
     expected offset field -> floor/frac (rounding-mode-agnostic).
  C. Phase-2 gather (dynamic idx): 14x14 CV windows + 14x448 qr0pad runs.
  D. Ref0: bilinear blend -> exp -> B-blur -> DVE contraction -> out0, Z0.
  E. Refs 1/2: transposed banded cc matmuls -> exp*mask -> PSUM-accumulated
     attention matmuls vs pre-transposed qr (ones channel = Z).
  F. Combine: (out12 + out0) / (Z12 + Z0) -> DRAM.
"""
from contextlib import ExitStack

import numpy as np
import ml_dtypes

import concourse.bass as bass

NPBF16 = ml_dtypes.bfloat16
import concourse.mybir as mybir
import concourse.tile as tile

F32 = mybir.dt.float32
I32 = mybir.dt.int32
BF16 = mybir.dt.bfloat16

# ---------------- geometry ----------------
D_SUB, R, C = 4, 6, 32
P13 = 2 * R + 1          # 13
N169 = P13 * P13
DIL_INT = 15
H = W = 56
CF = 64
NCORES = 8
RY = H // NCORES         # 7

DIL = 3
MAXOFF = R * DIL         # 18
DYLO = MAXOFF + R        # 24
NDY = 2 * DYLO + 2       # 50 rows needed for one y-row
WCV = W + NDY - 1        # 105
NROWS_G = NDY + 1        # 51 rows per pair group
HP = H + NDY - 1         # 105
WB = W + 2 * R           # 68
H_SLAB = H + 1           # 57: uniform 51-row pitch for all 4 groups
NRQ = RY + 2 * R         # 19
CC_RUN = 3 * (P13 - 1) + 1   # 37

FLOOR_BIAS = 1024.0
IDX_BIAS = int(FLOOR_BIAS) * WCV + int(FLOOR_BIAS)

GROUPS = [(0, 0, 128), (1, 2, 128), (2, 4, 128), (3, 6, 128)]
PPG = 128  # partitions per group: rows at offsets 0 and 64


def _pad2(a, top, left, hh, ww):
    out = np.zeros(a.shape[:-2] + (hh, ww), a.dtype)
    out[..., top:top + a.shape[-2], left:left + a.shape[-1]] = a
    return out


def host_prep(feats_r, feats_t, quantized_r, ref_index, current_ind):
    feats_r = np.asarray(feats_r, np.float32)
    feats_t = np.asarray(feats_t, np.float32)
    quantized_r = np.asarray(quantized_r, np.float32)
    ri = np.asarray(ref_index).tolist()
    ci = int(current_ind)
    diffs = [ci - int(x) for x in ri]
    nsearch = sum(1 for d in diffs if d > DIL_INT)
    dirates = [min(4, d // DIL_INT + 1) for d in diffs if d > DIL_INT]
    nref = feats_r.shape[0]
    assert nsearch == 1 and dirates[0] == DIL and nref == 3, \
        (nsearch, dirates, nref)

    f1 = feats_t[0]
    f2 = [feats_r[s, 0] for s in range(nref)]
    qr = [quantized_r[s, 0][:, ::D_SUB, ::D_SUB] for s in range(nref)]

    # row-interleaved qr0: QI[r, x, u, c] = qr0pad[r+u, x, c]
    qr0can = np.zeros((HP + 14, HP, C), np.float32)
    qr0can[DYLO:DYLO + H, DYLO:DYLO + W, :] = qr[0].transpose(1, 2, 0)
    qi = np.stack([qr0can[u:u + HP] for u in range(14)], axis=2)  # [HP,HP,14,C]
    qi = qi.reshape(HP * HP * 14 * C, 1)
    qi_b16 = np.ascontiguousarray(qi.astype(NPBF16))

    # f2_0 canvas: rows [-24 .. H+26], cols [-24 .. 80]
    f2p0 = _pad2(f2[0], DYLO, DYLO, H + 2 * DYLO + 3, WCV)
    f2p12 = [_pad2(f2[r], R, R, H + 2 * R, WB) for r in (1, 2)]
    qrpT = []
    for r in (1, 2):
        q = np.zeros((H + 2 * R, WB, C + 1), np.float32)
        q[R:R + H, R:R + W, :C] = qr[r].transpose(1, 2, 0)
        q[:, :, C] = 1.0
        qrpT.append(np.ascontiguousarray(q.transpose(1, 0, 2)))

    ploc128 = np.arange(PPG)
    yloc = (ploc128 >= 64).astype(np.int64)
    xs = np.minimum(ploc128 - 64 * yloc, W - 1)
    ploc = ploc128  # flat pixel slot in CV dram (includes dummy lanes)
    # phase-1 static stream into compact cvcc [p, 13, 105]: x window x+6
    sidx1 = ((ploc * P13) * WCV + xs + R)[:, None]
    # phase-2 CV stream const: row 18+yloc, col x+18; + oi_y*105 + oi_x
    c2cv = (((ploc * NROWS_G + MAXOFF + yloc) * WCV + xs + MAXOFF)
            - IDX_BIAS)[:, None]
    # phase-2 QI stream const (element units): ((y+18+yloc)*105 + x+18)*448
    c2qr = ((((yloc + MAXOFF) * WCV + xs + MAXOFF) - IDX_BIAS) * 448)[:, None]

    gridy = np.tile((np.repeat(np.arange(P13) - R, P13) * DIL)[None, :],
                    (PPG, 1)).astype(np.float32)
    gridx = np.tile((np.tile(np.arange(P13) - R, P13) * DIL)[None, :],
                    (PPG, 1)).astype(np.float32)

    xq = np.arange(WB)[:, None]
    xx = np.arange(W)[None, :]
    maskT = ((xq - xx >= 0) & (xq - xx <= 2 * R)).astype(np.float32)
    maskT_tiled = np.ascontiguousarray(
        np.tile(maskT[:, None, :], (1, P13, 1)).reshape(WB, P13 * W))

    def b16(a):
        return np.ascontiguousarray(a.astype(NPBF16))

    in_maps = []
    for core in range(NCORES):
        y0 = core * RY
        f1pair = np.zeros((CF, 4 * PPG), np.float32)
        for g in range(4):
            f1pair[:, g * PPG:g * PPG + W] = f1[:, y0 + 2 * g, :]
            if 2 * g + 1 < RY:
                f1pair[:, g * PPG + 64:g * PPG + 64 + W] = f1[:, y0 + 2 * g + 1, :]
        m = dict(
            f1=b16(f1[:, y0:y0 + RY, :].reshape(CF, RY * W)),
            f1pair=b16(f1pair),
            f2p0=b16(
                f2p0[:, y0:y0 + H_SLAB, :].reshape(CF, H_SLAB * WCV)),
            f2p1=b16(f2p12[0][:, y0:y0 + NRQ, :].reshape(CF, NRQ * WB)),
            f2p2=b16(f2p12[1][:, y0:y0 + NRQ, :].reshape(CF, NRQ * WB)),
            qrT1=b16(qrpT[0][:, y0:y0 + NRQ, :].reshape(WB, NRQ * (C + 1))),
            qrT2=b16(qrpT[1][:, y0:y0 + NRQ, :].reshape(WB, NRQ * (C + 1))),
            qr0pad=qi_b16,
            idx1=sidx1.astype(np.int32),
            c2cv=c2cv.astype(np.float32),
            c2qr=(c2qr + y0 * WCV * 448).astype(np.float32),
            gridx=gridx, gridy=gridy,
            maskT=b16(maskT_tiled),
        )
        in_maps.append(m)
    return in_maps


INPUT_SPECS = dict(
    f1=([CF, RY * W], BF16), f1pair=([CF, 4 * PPG], BF16),
    f2p0=([CF, H_SLAB * WCV], BF16),
    f2p1=([CF, NRQ * WB], BF16), f2p2=([CF, NRQ * WB], BF16),
    qrT1=([WB, NRQ * (C + 1)], BF16), qrT2=([WB, NRQ * (C + 1)], BF16),
    qr0pad=([HP * HP * 14 * C, 1], BF16),
    idx1=([PPG, 1], I32), c2cv=([PPG, 1], F32), c2qr=([PPG, 1], F32),
    gridx=([PPG, N169], F32), gridy=([PPG, N169], F32),
    maskT=([WB, P13 * W], BF16),
)
OUT_SPEC = ([RY * W, C], F32)


def build_kernel(tc, outs, ins):
    nc = tc.nc
    Exp = mybir.ActivationFunctionType.Exp
    ALU = mybir.AluOpType
    AX = mybir.AxisListType

    with ExitStack() as ctx:
        sb = ctx.enter_context(tc.tile_pool(name="sb", bufs=1))
        sbg = ctx.enter_context(tc.tile_pool(name="sbg", bufs=2))
        sbg3 = ctx.enter_context(tc.tile_pool(name="sbg3", bufs=3))
        ps_cv = ctx.enter_context(tc.tile_pool(name="ps_cv", bufs=2, space="PSUM"))
        ps_cc = ctx.enter_context(tc.tile_pool(name="ps_cc", bufs=2, space="PSUM"))
        ps_out = ctx.enter_context(tc.tile_pool(name="ps_out", bufs=2, space="PSUM"))
        dram = ctx.enter_context(tc.tile_pool(name="dram", bufs=1, space="DRAM"))

        def load(name, dtype=None):
            shape, dt_ = INPUT_SPECS[name]
            t = sb.tile(shape, dtype or dt_, tag=name)
            nc.sync.dma_start(t[:], ins[name])
            return t

        f1_t = load("f1")
        f1pair_t = load("f1pair")
        f2p0_t = load("f2p0")
        f2p12_t = [load("f2p1"), load("f2p2")]
        qrT_t = [load("qrT1"), load("qrT2")]
        idx1_t = load("idx1")
        c2cv_t = load("c2cv")
        c2qr_t = load("c2qr")
        gridx_t = load("gridx")
        gridy_t = load("gridy")
        maskT_t = load("maskT")

        out0_g, z0_g = {}, {}

        # ---------- E/F. refs 1/2 + combine (emitted per group) ----------
        def emit_row(yr):
            op = ps_out.tile([W, C + 1], F32, tag="op")
            first = True
            for r in (0, 1):
                em = sbg.tile([WB, P13 * W], BF16, tag="em")
                for h0, hn in ((0, 7), (7, 6)):
                    ct = ps_cc.tile([WB, 7 * W], F32, tag="ct")
                    for i in range(hn):
                        iy = h0 + i
                        nc.tensor.matmul(
                            ct[:, i * W:(i + 1) * W],
                            lhsT=f2p12_t[r][:, (yr + iy) * WB:(yr + iy + 1) * WB],
                            rhs=f1_t[:, yr * W:(yr + 1) * W],
                            start=True, stop=True)
                    nc.scalar.activation(em[:, h0 * W:(h0 + hn) * W],
                                         ct[:, 0:hn * W], Exp)
                nc.vector.tensor_tensor(em[:], em[:], maskT_t[:], op=ALU.mult)
                for iy in range(P13):
                    nc.tensor.matmul(
                        op[:], lhsT=em[:, iy * W:(iy + 1) * W],
                        rhs=qrT_t[r][:, (yr + iy) * (C + 1):
                                     (yr + iy + 1) * (C + 1)],
                        start=first, stop=(r == 1 and iy == P13 - 1))
                    first = False
            g = yr // 2
            p_lo = 64 * (yr % 2)
            psl = slice(p_lo, p_lo + W)
            den = sbg.tile([W, 2], F32, tag="den")
            nc.vector.tensor_tensor(den[:, 0:1], op[:, C:C + 1],
                                    z0_g[g][psl, N169:N169 + 1], op=ALU.add)
            nc.vector.reciprocal(den[:, 1:2], den[:, 0:1])
            of = sbg.tile([W, C], F32, tag="of")
            nc.vector.tensor_tensor(of[:], op[:, 0:C], out0_g[g][psl, :],
                                    op=ALU.add)
            nc.vector.tensor_scalar(of[:], of[:], den[:, 1:2], None,
                                    op0=ALU.mult)
            nc.sync.dma_start(
                outs["out"].rearrange("(y x) c -> y x c", y=RY)[yr], of[:])


        for g, yg, MP in GROUPS:
            nrow = NROWS_G
            # ---------- A. CV ----------
            cv_sb = sbg.tile([MP, nrow * WCV], BF16, tag="cv_sb")
            lhs = f1pair_t[:, g * PPG:(g + 1) * PPG]
            CH = 8
            ci = 0
            for r0 in range(0, nrow, CH):
                rn = min(CH, nrow - r0)
                pt = ps_cv.tile([MP, CH * 128], F32, tag="cvch")
                for r in range(rn):
                    row = 2 * g + r0 + r
                    nc.tensor.matmul(
                        pt[:, r * 128:r * 128 + WCV],
                        lhsT=lhs, rhs=f2p0_t[:, row * WCV:(row + 1) * WCV],
                        start=True, stop=True)
                dst = cv_sb[:, r0 * WCV:(r0 + rn) * WCV].rearrange(
                    "p (r w) -> p r w", r=rn)
                src = pt[:].rearrange("p (r w) -> p r w", r=CH)[:, 0:rn, 0:WCV]
                if ci % 2 == 0:
                    nc.vector.tensor_copy(dst, src)
                else:
                    nc.scalar.copy(dst, src)
                ci += 1
            cv_dram = dram.tile([MP * nrow * WCV, 1], BF16, tag=f"cvd{g}")
            nc.sync.dma_start(
                cv_dram[:].rearrange("(p f) o -> p (f o)", p=MP), cv_sb[:])
            # compact stride-3 rows for phase-1: cvcc [p, 13, 105]
            cvcc = dram.tile([MP * P13 * WCV, 1], BF16, tag=f"cvcc{g}")
            ccv_w = cvcc[:].rearrange("(p f) o -> p (f o)", p=MP).rearrange(
                "p (i w) -> p i w", i=P13)
            for pl, base in ((slice(0, 64), R), (slice(64, 128), R + 1)):
                sl = cv_sb[pl]
                src = bass.AP(sl.tensor, sl.offset + base * WCV,
                              [sl.ap[0], [3 * WCV, P13], [1, WCV]])
                nc.sync.dma_start(ccv_w[pl], src)

            # ---------- B. phase-1 (single stream per pixel, compact) ----------
            NS1 = 12 * WCV + CC_RUN
            g1 = sbg3.tile([MP, NS1], BF16, tag="g1")
            nc.gpsimd.indirect_dma_start(
                out=g1[:], out_offset=None, in_=cvcc[:],
                in_offset=bass.IndirectOffsetOnAxis(ap=idx1_t[0:MP, :], axis=0))
            cc0 = bass.AP(g1[:].tensor, g1[:].offset,
                          [g1[:].ap[0], [WCV, P13], [3, P13]])
            e1 = sbg.tile([MP, N169 + 1], F32, tag="e1")
            nc.scalar.activation(
                e1[:, 0:N169].rearrange("p (i j) -> p i j", i=P13), cc0, Exp,
                accum_out=e1[:, N169:N169 + 1])
            sc = sbg.tile([MP, 4], F32, tag="sc")
            tmp = sbg.tile([MP, N169], F32, tag="tmp169")
            nc.vector.scalar_tensor_tensor(
                out=tmp[:], in0=e1[:, 0:N169], scalar=0.0, in1=gridx_t[0:MP, :],
                op0=ALU.add, op1=ALU.mult, accum_out=sc[:, 0:1])
            nc.vector.scalar_tensor_tensor(
                out=tmp[:], in0=e1[:, 0:N169], scalar=0.0, in1=gridy_t[0:MP, :],
                op0=ALU.add, op1=ALU.mult, accum_out=sc[:, 1:2])
            offs = sbg.tile([MP, 2], F32, tag="offs")   # [off_x, off_y]
            nc.vector.reciprocal(sc[:, 2:3], e1[:, N169:N169 + 1])
            nc.vector.tensor_tensor(offs[:, 0:1], sc[:, 0:1], sc[:, 2:3],
                                    op=ALU.mult)
            nc.vector.tensor_tensor(offs[:, 1:2], sc[:, 1:2], sc[:, 2:3],
                                    op=ALU.mult)
            nc.vector.tensor_scalar(offs[:], offs[:], float(MAXOFF),
                                    -float(MAXOFF), op0=ALU.min, op1=ALU.max)
            # floor (mode-agnostic): fb = off+1024; fbi=cast; fbf=cast back;
            # fbf -= (fb - fbf < 0); wfrac = fb - fbf; fbi2 = cast(fbf)
            fb = sbg.tile([MP, 2], F32, tag="fb")
            nc.vector.tensor_scalar(fb[:], offs[:], FLOOR_BIAS, None,
                                    op0=ALU.add)
            fbi = sbg.tile([MP, 2], I32, tag="fbi")
            nc.vector.tensor_copy(fbi[:], fb[:])
            fbf = sbg.tile([MP, 2], F32, tag="fbf")
            nc.vector.tensor_copy(fbf[:], fbi[:])
            err = sbg.tile([MP, 2], F32, tag="err")
            nc.vector.tensor_tensor(err[:], fb[:], fbf[:], op=ALU.subtract)
            neg = sbg.tile([MP, 2], F32, tag="neg")
            nc.vector.tensor_scalar(neg[:], err[:], 0.0, None, op0=ALU.is_lt)
            nc.vector.tensor_tensor(fbf[:], fbf[:], neg[:], op=ALU.subtract)
            wfrac = sbg.tile([MP, 2], F32, tag="wfrac")  # [wx, wy]
            nc.vector.tensor_tensor(wfrac[:], fb[:], fbf[:], op=ALU.subtract)
            s2 = sbg.tile([MP, 1], F32, tag="s2")
            nc.vector.scalar_tensor_tensor(
                out=s2[:], in0=fbf[:, 1:2], scalar=float(WCV),
                in1=fbf[:, 0:1], op0=ALU.mult, op1=ALU.add)
            idx2cvf = sbg.tile([MP, 1], F32, tag="idx2cvf")
            nc.vector.tensor_scalar(idx2cvf[:], c2cv_t[0:MP, :], s2[:], None,
                                    op0=ALU.add)
            idx2cv = sbg.tile([MP, 1], I32, tag="idx2cv")
            nc.vector.tensor_copy(idx2cv[:], idx2cvf[:])
            # QI element index: c2qr + (s2 + yg*WCV)*448
            idx2qrf = sbg.tile([MP, 1], F32, tag="idx2qrf")
            nc.vector.tensor_scalar(idx2qrf[:], s2[:], 448.0,
                                    float(yg * WCV * 448),
                                    op0=ALU.mult, op1=ALU.add)
            nc.vector.tensor_tensor(idx2qrf[:], idx2qrf[:], c2qr_t[0:MP, :],
                                    op=ALU.add)
            idx2qr = sbg.tile([MP, 1], I32, tag="idx2qr")
            nc.vector.tensor_copy(idx2qr[:], idx2qrf[:])

            # ---------- C. phase-2 gathers (single stream per pixel) ----------
            NS2 = 13 * WCV + 14
            g2 = sbg3.tile([MP, NS2], BF16, tag="g2")
            nc.gpsimd.indirect_dma_start(
                out=g2[:], out_offset=None, in_=cv_dram[:],
                in_offset=bass.IndirectOffsetOnAxis(ap=idx2cv[:], axis=0))
            qt = sbg3.tile([MP, 14 * 448], BF16, tag="qt")
            nc.gpsimd.indirect_dma_start(
                out=qt[:], out_offset=None, in_=ins["qr0pad"],
                in_offset=bass.IndirectOffsetOnAxis(ap=idx2qr[:], axis=0))

            # ---------- D. ref0 ----------
            ww = sbg.tile([MP, 4], F32, tag="ww")
            om = sbg.tile([MP, 2], F32, tag="om")
            nc.vector.tensor_scalar(om[:], wfrac[:], -1.0, 1.0,
                                    op0=ALU.mult, op1=ALU.add)
            nc.vector.tensor_tensor(ww[:, 0:1], om[:, 1:2], om[:, 0:1],
                                    op=ALU.mult)
            nc.vector.tensor_tensor(ww[:, 1:2], om[:, 1:2], wfrac[:, 0:1],
                                    op=ALU.mult)
            nc.vector.tensor_tensor(ww[:, 2:3], wfrac[:, 1:2], om[:, 0:1],
                                    op=ALU.mult)
            nc.vector.tensor_tensor(ww[:, 3:4], wfrac[:, 1:2], wfrac[:, 0:1],
                                    op=ALU.mult)

            g2v = bass.AP(g2[:].tensor, g2[:].offset,
                          [g2[:].ap[0], [WCV, 14], [1, 14]])
            corr = sbg.tile([MP, N169], F32, tag="corr")
            crv = corr[:].rearrange("p (i j) -> p i j", i=P13)
            nc.vector.tensor_scalar(crv, g2v[:, 0:13, 0:13], ww[:, 0:1], None,
                                    op0=ALU.mult)
            for (sl_u, sl_v, wcol) in (((0, 13), (1, 14), 1),
                                       ((1, 14), (0, 13), 2),
                                       ((1, 14), (1, 14), 3)):
                nc.vector.scalar_tensor_tensor(
                    out=crv, in0=g2v[:, sl_u[0]:sl_u[1], sl_v[0]:sl_v[1]],
                    scalar=ww[:, wcol:wcol + 1], in1=crv,
                    op0=ALU.mult, op1=ALU.add)
            p0 = sbg.tile([MP, N169 + 1], F32, tag="p0")
            nc.scalar.activation(p0[:, 0:N169], corr[:], Exp,
                                 accum_out=p0[:, N169:N169 + 1])
            z0_g[g] = p0
            bb = sbg.tile([MP, 196], BF16, tag="bb")
            nc.vector.memset(bb[:], 0.0)
            bbv = bb[:].rearrange("p (u v) -> p u v", u=14)
            p0v = p0[:, 0:N169].rearrange("p (i j) -> p i j", i=P13)
            nc.vector.tensor_scalar(bbv[:, 0:13, 0:13], p0v, ww[:, 0:1], None,
                                    op0=ALU.mult)
            for (sl_u, sl_v, wcol) in (((0, 13), (1, 14), 1),
                                       ((1, 14), (0, 13), 2),
                                       ((1, 14), (1, 14), 3)):
                dstv = bbv[:, sl_u[0]:sl_u[1], sl_v[0]:sl_v[1]]
                nc.vector.scalar_tensor_tensor(
                    out=dstv, in0=p0v, scalar=ww[:, wcol:wcol + 1], in1=dstv,
                    op0=ALU.mult, op1=ALU.add)
            qtv = bass.AP(qt[:].tensor, qt[:].offset,
                          [qt[:].ap[0], [32, 14], [448, 14], [1, C]])  # (u,v,c)
            bbb = bbv.to_broadcast([MP, 14, 14, C])
            nc.vector.tensor_tensor(qtv, qtv, bbb, op=ALU.mult)
            o0 = sbg.tile([MP, C], F32, tag="o0")
            pr = bass.AP(qt[:].tensor, qt[:].offset,
                         [qt[:].ap[0], [1, C], [32, 14], [448, 14]])
            nc.vector.tensor_reduce(o0[:], pr, axis=AX.XY, op=ALU.add)
            out0_g[g] = o0
            for yr in (2 * g, 2 * g + 1):
                if yr < RY:
                    emit_row(yr)
            if g == 0 and "dbg_qt" in outs:
                nc.sync.dma_start(outs["dbg_qt"], qt[:])
                nc.sync.dma_start(outs["dbg_g2"], g2[:])
                nc.sync.dma_start(outs["dbg_idx2qr"], idx2qr[:])
                nc.sync.dma_start(outs["dbg_idx2cv"], idx2cv[:])
                nc.sync.dma_start(outs["dbg_o0"], o0[:])
                nc.sync.dma_start(outs["dbg_bb"], bb[:])
                nc.sync.dma_start(outs["dbg_offs"], offs[:])
                nc.sync.dma_start(outs["dbg_e1"], e1[:])
                nc.sync.dma_start(outs["dbg_g1"], g1[:])




# ---------------- numpy mirror of one core (debug) ----------------
def core_reference(m):
    m = {k: (np.asarray(v, np.float32) if v.dtype != np.int32 else v)
         for k, v in m.items()}
    f1 = m["f1"].reshape(CF, RY, W)
    f2p0 = m["f2p0"].reshape(CF, H_SLAB, WCV)
    o0full = np.zeros((RY, W, C), np.float32)
    z0full = np.zeros((RY, W, 1), np.float32)
    for g, yg, MP in GROUPS:
        nrow = NROWS_G
        lhs = m["f1pair"][:, g * PPG:(g + 1) * PPG].astype(np.float32)
        cv = np.einsum('cp,crw->prw', lhs, f2p0[:, 2 * g:2 * g + nrow, :])
        cvf = np.ascontiguousarray(cv).reshape(-1)
        yy = (np.arange(MP) >= 64).astype(np.int64)
        rows = (R + yy[:, None, None] + 3 * np.arange(P13)[None, :, None])
        cvcc = np.take_along_axis(
            cv, np.broadcast_to(rows, (MP, P13, WCV)), axis=1)
        ccf = np.ascontiguousarray(cvcc).reshape(-1)
        sidx = m["idx1"][:MP, 0] - (np.arange(MP) * NROWS_G - np.arange(MP) * P13) * WCV
        # device sidx indexes cvcc directly: (p*13)*WCV + x + 6
        sidx = (np.arange(MP) * P13 * WCV
                + np.minimum(np.arange(MP) - 64 * yy, W - 1) + R)
        NS1 = 12 * WCV + CC_RUN
        g1 = ccf[sidx[:, None] + np.arange(NS1)[None, :]]
        cc0 = np.stack([g1[:, i * WCV + 3 * np.arange(P13)]
                        for i in range(P13)], 1).reshape(MP, N169)
        e1 = np.exp(cc0)
        S = e1.sum(1, keepdims=True)
        offx = np.clip((e1 * m["gridx"][:MP]).sum(1, keepdims=True) / S,
                       -MAXOFF, MAXOFF)
        offy = np.clip((e1 * m["gridy"][:MP]).sum(1, keepdims=True) / S,
                       -MAXOFF, MAXOFF)
        fbx = np.floor(offx + FLOOR_BIAS)
        fby = np.floor(offy + FLOOR_BIAS)
        wx = (offx + FLOOR_BIAS) - fbx
        wy = (offy + FLOOR_BIAS) - fby
        s2 = (fby.astype(np.int64) * WCV + fbx.astype(np.int64))
        idx2cv = (m["c2cv"][:MP, 0] + s2[:, 0]).astype(np.int64)
        idx2qr = (m["c2qr"][:MP, 0] + (s2[:, 0] + yg * WCV) * 448).astype(np.int64)
        NS2 = 13 * WCV + 14
        g2s = cvf[idx2cv[:, None] + np.arange(NS2)[None, :]]
        g2 = np.stack([g2s[:, u * WCV:u * WCV + 14] for u in range(14)], 1)
        qrf = m["qr0pad"].reshape(-1)
        qts = qrf[idx2qr[:, None] + np.arange(6272)[None, :]]
        # stream order (v, u, c) -> [MP, u, v, c]
        qt = qts.reshape(MP, 14, 14, C).transpose(0, 2, 1, 3)
        w00 = (1 - wy) * (1 - wx); w01 = (1 - wy) * wx
        w10 = wy * (1 - wx); w11 = wy * wx
        corr = (w00 * g2[:, 0:13, 0:13].reshape(MP, N169)
                + w01 * np.ascontiguousarray(g2[:, 0:13, 1:14]).reshape(MP, N169)
                + w10 * np.ascontiguousarray(g2[:, 1:14, 0:13]).reshape(MP, N169)
                + w11 * np.ascontiguousarray(g2[:, 1:14, 1:14]).reshape(MP, N169))
        p0 = np.exp(corr)
        z0 = p0.sum(1, keepdims=True)
        bb = np.zeros((MP, 14, 14), np.float32)
        p0v = p0.reshape(MP, P13, P13)
        bb[:, 0:13, 0:13] += w00[..., None] * p0v
        bb[:, 0:13, 1:14] += w01[..., None] * p0v
        bb[:, 1:14, 0:13] += w10[..., None] * p0v
        bb[:, 1:14, 1:14] += w11[..., None] * p0v
        o0 = (qt * bb[..., None]).sum((1, 2))
        for yloc in range(2):
            if yg + yloc >= RY:
                continue
            o0full[yg + yloc] = o0[64 * yloc:64 * yloc + W]
            z0full[yg + yloc] = z0[64 * yloc:64 * yloc + W]
    out = np.zeros((RY, W, C), np.float32)
    maskT = m["maskT"].reshape(WB, P13, W)[:, 0, :]
    for yr in range(RY):
        acc = np.zeros((W, C + 1), np.float32)
        for r in range(2):
            f2p = m[f"f2p{r + 1}"].reshape(CF, NRQ, WB)
            qrT = m[f"qrT{r + 1}"].reshape(WB, NRQ, C + 1)
            for iy in range(P13):
                ct = np.einsum('cq,cx->qx', f2p[:, yr + iy, :], f1[:, yr, :])
                em = np.exp(ct) * maskT
                acc += np.einsum('qx,qd->xd', em, qrT[:, yr + iy, :])
        den = acc[:, C:C + 1] + z0full[yr]
        out[yr] = (acc[:, :C] + o0full[yr]) / den
    return out


def full_reference_from_cores(in_maps):
    outs = [core_reference(in_maps[i]) for i in range(NCORES)]
    full = np.stack(outs, 0)            # [8, 7, 56, C]
    return full.reshape(H, W, C).transpose(2, 0, 1)[None]


DEBUG_SPECS = dict(
    dbg_qt=([128, 14 * 448], F32), dbg_g2=([128, 196], F32),
    dbg_idx2qr=([128, 14], I32), dbg_idx2cv=([128, 14], I32),
    dbg_o0=([128, C], F32), dbg_bb=([128, 196], F32),
    dbg_offs=([128, 2], F32), dbg_e1=([128, N169 + 1], F32),
    dbg_g1=([128, P13 * CC_RUN], F32),
)


def build_program(ncores=NCORES, debug=False):
    import concourse.bacc as bacc
    nc = bacc.Bacc("TRN2", target_bir_lowering=False, debug=False,
                   enable_asserts=True, num_devices=ncores)
    ins = {}
    for name, (shape, dt_) in INPUT_SPECS.items():
        ins[name] = nc.dram_tensor(name, shape, dt_, kind="ExternalInput").ap()
    outs = {"out": nc.dram_tensor("out", OUT_SPEC[0], OUT_SPEC[1],
                                  kind="ExternalOutput").ap()}
    if debug:
        for name, (shape, dt_) in DEBUG_SPECS.items():
            outs[name] = nc.dram_tensor(name, shape, dt_,
                                        kind="ExternalOutput").ap()
    with tile.TileContext(nc) as tc:
        build_kernel(tc, outs, ins)
    nc.compile()
    return nc


# ======================= runner =======================
import os as _os


def _build_program():
    import concourse.bacc as bacc
    nc = bacc.Bacc("TRN2", target_bir_lowering=False, debug=False,
                   enable_asserts=True, num_devices=NCORES)
    ins = {}
    for name, (shape, dt_) in INPUT_SPECS.items():
        ins[name] = nc.dram_tensor(name, shape, dt_, kind="ExternalInput").ap()
    outs = {"out": nc.dram_tensor("out", OUT_SPEC[0], OUT_SPEC[1],
                                  kind="ExternalOutput").ap()}
    with tile.TileContext(nc) as tc:
        build_kernel(tc, outs, ins)
    nc.compile()
    return nc


_LAST_RESULT = {}


def kernel(**inputs):
    from concourse.bass_utils import run_bass_kernel_spmd
    from concourse.bass_interp import get_hw_module

    in_maps = host_prep(**inputs)
    nc = _build_program()
    nc.m = get_hw_module(nc.m)
    trace = _os.environ.get("KERNEL_TRACE", "0") == "1"
    res = run_bass_kernel_spmd(
        nc, in_maps, core_ids=list(range(NCORES)), trace=trace)
    _LAST_RESULT["res"] = res
    slabs = [np.asarray(res.results[i]["out"], np.float32).reshape(RY, W, C)
             for i in range(NCORES)]
    full = np.concatenate(slabs, 0)          # [56, 56, 32]
    return np.ascontiguousarray(full.transpose(2, 0, 1)[None])



# revision 6
# speedup vs baseline: 21.9971x; 21.9971x over previous
"""Bass/Tile kernel for nn_Colorizer (sparse deformable attention colorizer).

Sharding: spatial row-sharding across 8 cores; core i owns output rows
[7i, 7i+7). All refs computed on every core for its rows; the final joint
softmax is additive across refs so each core normalizes locally.

Per-core pipeline:
  A. CV volume (search ref): banded PE matmuls -> CV[pixel, row, dx(105)]
     per pair-group -> SBUF -> DRAM.
  B. Phase-1 gather (static idx): stride-3 rows of CV -> cc0 -> exp ->
     expected offset field -> floor/frac (rounding-mode-agnostic).
  C. Phase-2 gather (dynamic idx): 14x14 CV windows + 14x448 qr0pad runs.
  D. Ref0: bilinear blend -> exp -> B-blur -> DVE contraction -> out0, Z0.
  E. Refs 1/2: transposed banded cc matmuls -> exp*mask -> PSUM-accumulated
     attention matmuls vs pre-transposed qr (ones channel = Z).
  F. Combine: (out12 + out0) / (Z12 + Z0) -> DRAM.
"""
from contextlib import ExitStack

import numpy as np
import ml_dtypes

import concourse.bass as bass

NPBF16 = ml_dtypes.bfloat16
import concourse.mybir as mybir
import concourse.tile as tile

F32 = mybir.dt.float32
I32 = mybir.dt.int32
BF16 = mybir.dt.bfloat16

# ---------------- geometry ----------------
D_SUB, R, C = 4, 6, 32
P13 = 2 * R + 1          # 13
N169 = P13 * P13
DIL_INT = 15
H = W = 56
CF = 64
NCORES = 8
RY = H // NCORES         # 7

DIL = 3
MAXOFF = R * DIL         # 18
DYLO = MAXOFF + R        # 24
NDY = 2 * DYLO + 2       # 50 rows needed for one y-row
WCV = W + NDY - 1        # 105
NROWS_G = NDY + 1        # 51 rows per pair group
HP = H + NDY - 1         # 105
WB = W + 2 * R           # 68
H_SLAB = H + 1           # 57: uniform 51-row pitch for all 4 groups
NRQ = RY + 2 * R         # 19
CC_RUN = 3 * (P13 - 1) + 1   # 37

FLOOR_BIAS = 1024.0
IDX_BIAS = int(FLOOR_BIAS) * WCV + int(FLOOR_BIAS)

GROUPS = [(0, 0, 128), (1, 2, 128), (2, 4, 128), (3, 6, 128)]
PPG = 128  # partitions per group: rows at offsets 0 and 64


def _pad2(a, top, left, hh, ww):
    out = np.zeros(a.shape[:-2] + (hh, ww), a.dtype)
    out[..., top:top + a.shape[-2], left:left + a.shape[-1]] = a
    return out


def host_prep(feats_r, feats_t, quantized_r, ref_index, current_ind):
    feats_r = np.asarray(feats_r, np.float32)
    feats_t = np.asarray(feats_t, np.float32)
    quantized_r = np.asarray(quantized_r, np.float32)
    ri = np.asarray(ref_index).tolist()
    ci = int(current_ind)
    diffs = [ci - int(x) for x in ri]
    nsearch = sum(1 for d in diffs if d > DIL_INT)
    dirates = [min(4, d // DIL_INT + 1) for d in diffs if d > DIL_INT]
    nref = feats_r.shape[0]
    assert nsearch == 1 and dirates[0] == DIL and nref == 3, \
        (nsearch, dirates, nref)

    f1 = feats_t[0]
    f2 = [feats_r[s, 0] for s in range(nref)]
    qr = [quantized_r[s, 0][:, ::D_SUB, ::D_SUB] for s in range(nref)]

    # row-interleaved qr0: QI[r, x, u, c] = qr0pad[r+u, x, c]
    qr0can = np.zeros((HP + 14, HP, C), np.float32)
    qr0can[DYLO:DYLO + H, DYLO:DYLO + W, :] = qr[0].transpose(1, 2, 0)
    qi = np.stack([qr0can[u:u + HP] for u in range(14)], axis=2)  # [HP,HP,14,C]
    qi = qi.reshape(1, HP * HP * 14 * C)
    qi_b16 = np.ascontiguousarray(qi.astype(NPBF16))

    # f2_0 canvas: rows [-24 .. H+26], cols [-24 .. 80]
    f2p0 = _pad2(f2[0], DYLO, DYLO, H + 2 * DYLO + 3, WCV)
    f2p12 = [_pad2(f2[r], R, R, H + 2 * R, WB) for r in (1, 2)]
    qrpT = []
    for r in (1, 2):
        q = np.zeros((H + 2 * R, WB, C + 1), np.float32)
        q[R:R + H, R:R + W, :C] = qr[r].transpose(1, 2, 0)
        q[:, :, C] = 1.0
        qrpT.append(np.ascontiguousarray(q.transpose(1, 0, 2)))

    ploc128 = np.arange(PPG)
    yloc = (ploc128 >= 64).astype(np.int64)
    xs = np.minimum(ploc128 - 64 * yloc, W - 1)
    ploc = ploc128  # flat pixel slot in CV dram (includes dummy lanes)
    # phase-1 static stream into compact cvcc [p, 13, 105]: x window x+6
    sidx1 = ((ploc * P13) * WCV + xs + R)[:, None]
    # phase-2 CV stream const: row 18+yloc, col x+18; + oi_y*105 + oi_x
    c2cv = (((ploc * NROWS_G + MAXOFF + yloc) * WCV + xs + MAXOFF)
            - IDX_BIAS)[:, None]
    # phase-2 QI stream const (element units): ((y+18+yloc)*105 + x+18)*448
    c2qr = ((((yloc + MAXOFF) * WCV + xs + MAXOFF) - IDX_BIAS) * 448)[:, None]

    gridy = np.tile((np.repeat(np.arange(P13) - R, P13) * DIL)[None, :],
                    (PPG, 1)).astype(np.float32)
    gridx = np.tile((np.tile(np.arange(P13) - R, P13) * DIL)[None, :],
                    (PPG, 1)).astype(np.float32)

    xq = np.arange(WB)[:, None]
    xx = np.arange(W)[None, :]
    maskT = ((xq - xx >= 0) & (xq - xx <= 2 * R)).astype(np.float32)
    maskT_tiled = np.ascontiguousarray(
        np.tile(maskT[:, None, :], (1, P13, 1)).reshape(WB, P13 * W))

    def b16(a):
        return np.ascontiguousarray(a.astype(NPBF16))

    in_maps = []
    for core in range(NCORES):
        y0 = core * RY
        f1pair = np.zeros((CF, 4 * PPG), np.float32)
        for g in range(4):
            f1pair[:, g * PPG:g * PPG + W] = f1[:, y0 + 2 * g, :]
            if 2 * g + 1 < RY:
                f1pair[:, g * PPG + 64:g * PPG + 64 + W] = f1[:, y0 + 2 * g + 1, :]
        m = dict(
            f1=b16(f1[:, y0:y0 + RY, :].reshape(CF, RY * W)),
            f1pair=b16(f1pair),
            f2p0=b16(
                f2p0[:, y0:y0 + H_SLAB, :].reshape(CF, H_SLAB * WCV)),
            f2p1=b16(f2p12[0][:, y0:y0 + NRQ, :].reshape(CF, NRQ * WB)),
            f2p2=b16(f2p12[1][:, y0:y0 + NRQ, :].reshape(CF, NRQ * WB)),
            qrT1=b16(qrpT[0][:, y0:y0 + NRQ, :].reshape(WB, NRQ * (C + 1))),
            qrT2=b16(qrpT[1][:, y0:y0 + NRQ, :].reshape(WB, NRQ * (C + 1))),
            qr0pad=qi_b16,
            idx1=sidx1.astype(np.int32),
            c2cv=c2cv.astype(np.float32),
            c2qr=(c2qr + y0 * WCV * 448).astype(np.float32),
            gridx=gridx, gridy=gridy,
            maskT=b16(maskT_tiled),
        )
        in_maps.append(m)
    return in_maps


INPUT_SPECS = dict(
    f1=([CF, RY * W], BF16), f1pair=([CF, 4 * PPG], BF16),
    f2p0=([CF, H_SLAB * WCV], BF16),
    f2p1=([CF, NRQ * WB], BF16), f2p2=([CF, NRQ * WB], BF16),
    qrT1=([WB, NRQ * (C + 1)], BF16), qrT2=([WB, NRQ * (C + 1)], BF16),
    qr0pad=([1, HP * HP * 14 * C], BF16),
    idx1=([PPG, 1], I32), c2cv=([PPG, 1], F32), c2qr=([PPG, 1], F32),
    gridx=([PPG, N169], F32), gridy=([PPG, N169], F32),
    maskT=([WB, P13 * W], BF16),
)
OUT_SPEC = ([RY * W, C], F32)


def build_kernel(tc, outs, ins):
    nc = tc.nc
    Exp = mybir.ActivationFunctionType.Exp
    ALU = mybir.AluOpType
    AX = mybir.AxisListType

    with ExitStack() as ctx:
        sb = ctx.enter_context(tc.tile_pool(name="sb", bufs=1))
        sbg = ctx.enter_context(tc.tile_pool(name="sbg", bufs=2))
        sbg3 = ctx.enter_context(tc.tile_pool(name="sbg3", bufs=3))
        ps_cv = ctx.enter_context(tc.tile_pool(name="ps_cv", bufs=2, space="PSUM"))
        ps_cc = ctx.enter_context(tc.tile_pool(name="ps_cc", bufs=2, space="PSUM"))
        ps_out = ctx.enter_context(tc.tile_pool(name="ps_out", bufs=2, space="PSUM"))
        dram = ctx.enter_context(tc.tile_pool(name="dram", bufs=1, space="DRAM"))

        def load(name, dtype=None):
            shape, dt_ = INPUT_SPECS[name]
            t = sb.tile(shape, dtype or dt_, tag=name)
            nc.sync.dma_start(t[:], ins[name])
            return t

        f1_t = load("f1")
        f1pair_t = load("f1pair")
        f2p0_t = load("f2p0")
        f2p12_t = [load("f2p1"), load("f2p2")]
        qrT_t = [load("qrT1"), load("qrT2")]
        idx1_t = load("idx1")
        c2cv_t = load("c2cv")
        c2qr_t = load("c2qr")
        gridx_t = load("gridx")
        gridy_t = load("gridy")
        maskT_t = load("maskT")

        out0_g, z0_g = {}, {}

        # ---------- E/F. refs 1/2 + combine (emitted per group) ----------
        def emit_row(yr):
            op = ps_out.tile([W, C + 1], F32, tag="op")
            first = True
            for r in (0, 1):
                em = sbg.tile([WB, P13 * W], BF16, tag="em")
                for h0, hn in ((0, 7), (7, 6)):
                    ct = ps_cc.tile([WB, 7 * W], F32, tag="ct")
                    for i in range(hn):
                        iy = h0 + i
                        nc.tensor.matmul(
                            ct[:, i * W:(i + 1) * W],
                            lhsT=f2p12_t[r][:, (yr + iy) * WB:(yr + iy + 1) * WB],
                            rhs=f1_t[:, yr * W:(yr + 1) * W],
                            start=True, stop=True)
                    nc.scalar.activation(em[:, h0 * W:(h0 + hn) * W],
                                         ct[:, 0:hn * W], Exp)
                nc.vector.tensor_tensor(em[:], em[:], maskT_t[:], op=ALU.mult)
                for iy in range(P13):
                    nc.tensor.matmul(
                        op[:], lhsT=em[:, iy * W:(iy + 1) * W],
                        rhs=qrT_t[r][:, (yr + iy) * (C + 1):
                                     (yr + iy + 1) * (C + 1)],
                        start=first, stop=(r == 1 and iy == P13 - 1))
                    first = False
            g = yr // 2
            p_lo = 64 * (yr % 2)
            psl = slice(p_lo, p_lo + W)
            den = sbg.tile([W, 2], F32, tag="den")
            nc.vector.tensor_tensor(den[:, 0:1], op[:, C:C + 1],
                                    z0_g[g][psl, N169:N169 + 1], op=ALU.add)
            nc.vector.reciprocal(den[:, 1:2], den[:, 0:1])
            of = sbg.tile([W, C], F32, tag="of")
            nc.vector.tensor_tensor(of[:], op[:, 0:C], out0_g[g][psl, :],
                                    op=ALU.add)
            nc.vector.tensor_scalar(of[:], of[:], den[:, 1:2], None,
                                    op0=ALU.mult)
            nc.sync.dma_start(
                outs["out"].rearrange("(y x) c -> y x c", y=RY)[yr], of[:])


        for g, yg, MP in GROUPS:
            nrow = NROWS_G
            # ---------- A. CV ----------
            cv_sb = sbg.tile([MP, nrow * WCV], BF16, tag="cv_sb")
            lhs = f1pair_t[:, g * PPG:(g + 1) * PPG]
            CH = 8
            ci = 0
            for r0 in range(0, nrow, CH):
                rn = min(CH, nrow - r0)
                pt = ps_cv.tile([MP, CH * 128], F32, tag="cvch")
                for r in range(rn):
                    row = 2 * g + r0 + r
                    nc.tensor.matmul(
                        pt[:, r * 128:r * 128 + WCV],
                        lhsT=lhs, rhs=f2p0_t[:, row * WCV:(row + 1) * WCV],
                        start=True, stop=True)
                dst = cv_sb[:, r0 * WCV:(r0 + rn) * WCV].rearrange(
                    "p (r w) -> p r w", r=rn)
                src = pt[:].rearrange("p (r w) -> p r w", r=CH)[:, 0:rn, 0:WCV]
                if ci % 2 == 0:
                    nc.vector.tensor_copy(dst, src)
                else:
                    nc.scalar.copy(dst, src)
                ci += 1
            # [1, X] shape: keeps the cost model's descriptor granularity at
            # one contiguous run per gather index instead of per element.
            cv_dram = dram.tile([1, MP * nrow * WCV], BF16, tag=f"cvd{g}")
            nc.sync.dma_start(
                cv_dram[:].rearrange("o (p f) -> p (f o)", p=MP), cv_sb[:])
            # compact stride-3 rows for phase-1: cvcc [p, 13, 105]
            cvcc = dram.tile([1, MP * P13 * WCV], BF16, tag=f"cvcc{g}")
            ccv_w = cvcc[:].rearrange("o (p f) -> p (f o)", p=MP).rearrange(
                "p (i w) -> p i w", i=P13)
            for pl, base in ((slice(0, 64), R), (slice(64, 128), R + 1)):
                sl = cv_sb[pl]
                src = bass.AP(sl.tensor, sl.offset + base * WCV,
                              [sl.ap[0], [3 * WCV, P13], [1, WCV]])
                nc.sync.dma_start(ccv_w[pl], src)

            # ---------- B. phase-1 (single stream per pixel, compact) ----------
            NS1 = 12 * WCV + CC_RUN
            g1 = sbg3.tile([MP, NS1], BF16, tag="g1")
            nc.gpsimd.indirect_dma_start(
                out=g1[:], out_offset=None, in_=cvcc[:],
                in_offset=bass.IndirectOffsetOnAxis(ap=idx1_t[0:MP, :], axis=1))
            cc0 = bass.AP(g1[:].tensor, g1[:].offset,
                          [g1[:].ap[0], [WCV, P13], [3, P13]])
            e1 = sbg.tile([MP, N169 + 1], F32, tag="e1")
            nc.scalar.activation(
                e1[:, 0:N169].rearrange("p (i j) -> p i j", i=P13), cc0, Exp,
                accum_out=e1[:, N169:N169 + 1])
            sc = sbg.tile([MP, 4], F32, tag="sc")
            tmp = sbg.tile([MP, N169], F32, tag="tmp169")
            nc.vector.scalar_tensor_tensor(
                out=tmp[:], in0=e1[:, 0:N169], scalar=0.0, in1=gridx_t[0:MP, :],
                op0=ALU.add, op1=ALU.mult, accum_out=sc[:, 0:1])
            nc.vector.scalar_tensor_tensor(
                out=tmp[:], in0=e1[:, 0:N169], scalar=0.0, in1=gridy_t[0:MP, :],
                op0=ALU.add, op1=ALU.mult, accum_out=sc[:, 1:2])
            offs = sbg.tile([MP, 2], F32, tag="offs")   # [off_x, off_y]
            nc.vector.reciprocal(sc[:, 2:3], e1[:, N169:N169 + 1])
            nc.vector.tensor_tensor(offs[:, 0:1], sc[:, 0:1], sc[:, 2:3],
                                    op=ALU.mult)
            nc.vector.tensor_tensor(offs[:, 1:2], sc[:, 1:2], sc[:, 2:3],
                                    op=ALU.mult)
            nc.vector.tensor_scalar(offs[:], offs[:], float(MAXOFF),
                                    -float(MAXOFF), op0=ALU.min, op1=ALU.max)
            # floor (mode-agnostic): fb = off+1024; fbi=cast; fbf=cast back;
            # fbf -= (fb - fbf < 0); wfrac = fb - fbf; fbi2 = cast(fbf)
            fb = sbg.tile([MP, 2], F32, tag="fb")
            nc.vector.tensor_scalar(fb[:], offs[:], FLOOR_BIAS, None,
                                    op0=ALU.add)
            fbi = sbg.tile([MP, 2], I32, tag="fbi")
            nc.vector.tensor_copy(fbi[:], fb[:])
            fbf = sbg.tile([MP, 2], F32, tag="fbf")
            nc.vector.tensor_copy(fbf[:], fbi[:])
            err = sbg.tile([MP, 2], F32, tag="err")
            nc.vector.tensor_tensor(err[:], fb[:], fbf[:], op=ALU.subtract)
            neg = sbg.tile([MP, 2], F32, tag="neg")
            nc.vector.tensor_scalar(neg[:], err[:], 0.0, None, op0=ALU.is_lt)
            nc.vector.tensor_tensor(fbf[:], fbf[:], neg[:], op=ALU.subtract)
            wfrac = sbg.tile([MP, 2], F32, tag="wfrac")  # [wx, wy]
            nc.vector.tensor_tensor(wfrac[:], fb[:], fbf[:], op=ALU.subtract)
            s2 = sbg.tile([MP, 1], F32, tag="s2")
            nc.vector.scalar_tensor_tensor(
                out=s2[:], in0=fbf[:, 1:2], scalar=float(WCV),
                in1=fbf[:, 0:1], op0=ALU.mult, op1=ALU.add)
            idx2cvf = sbg.tile([MP, 1], F32, tag="idx2cvf")
            nc.vector.tensor_scalar(idx2cvf[:], c2cv_t[0:MP, :], s2[:], None,
                                    op0=ALU.add)
            idx2cv = sbg.tile([MP, 1], I32, tag="idx2cv")
            nc.vector.tensor_copy(idx2cv[:], idx2cvf[:])
            # QI element index: c2qr + (s2 + yg*WCV)*448
            idx2qrf = sbg.tile([MP, 1], F32, tag="idx2qrf")
            nc.vector.tensor_scalar(idx2qrf[:], s2[:], 448.0,
                                    float(yg * WCV * 448),
                                    op0=ALU.mult, op1=ALU.add)
            nc.vector.tensor_tensor(idx2qrf[:], idx2qrf[:], c2qr_t[0:MP, :],
                                    op=ALU.add)
            idx2qr = sbg.tile([MP, 1], I32, tag="idx2qr")
            nc.vector.tensor_copy(idx2qr[:], idx2qrf[:])

            # ---------- C. phase-2 gathers (single stream per pixel) ----------
            NS2 = 13 * WCV + 14
            g2 = sbg3.tile([MP, NS2], BF16, tag="g2")
            nc.gpsimd.indirect_dma_start(
                out=g2[:], out_offset=None, in_=cv_dram[:],
                in_offset=bass.IndirectOffsetOnAxis(ap=idx2cv[:], axis=1))
            qt = sbg3.tile([MP, 14 * 448], BF16, tag="qt")
            nc.gpsimd.indirect_dma_start(
                out=qt[:], out_offset=None, in_=ins["qr0pad"],
                in_offset=bass.IndirectOffsetOnAxis(ap=idx2qr[:], axis=1))

            # ---------- D. ref0 ----------
            ww = sbg.tile([MP, 4], F32, tag="ww")
            om = sbg.tile([MP, 2], F32, tag="om")
            nc.vector.tensor_scalar(om[:], wfrac[:], -1.0, 1.0,
                                    op0=ALU.mult, op1=ALU.add)
            nc.vector.tensor_tensor(ww[:, 0:1], om[:, 1:2], om[:, 0:1],
                                    op=ALU.mult)
            nc.vector.tensor_tensor(ww[:, 1:2], om[:, 1:2], wfrac[:, 0:1],
                                    op=ALU.mult)
            nc.vector.tensor_tensor(ww[:, 2:3], wfrac[:, 1:2], om[:, 0:1],
                                    op=ALU.mult)
            nc.vector.tensor_tensor(ww[:, 3:4], wfrac[:, 1:2], wfrac[:, 0:1],
                                    op=ALU.mult)

            g2v = bass.AP(g2[:].tensor, g2[:].offset,
                          [g2[:].ap[0], [WCV, 14], [1, 14]])
            corr = sbg.tile([MP, N169], F32, tag="corr")
            crv = corr[:].rearrange("p (i j) -> p i j", i=P13)
            nc.vector.tensor_scalar(crv, g2v[:, 0:13, 0:13], ww[:, 0:1], None,
                                    op0=ALU.mult)
            for (sl_u, sl_v, wcol) in (((0, 13), (1, 14), 1),
                                       ((1, 14), (0, 13), 2),
                                       ((1, 14), (1, 14), 3)):
                nc.vector.scalar_tensor_tensor(
                    out=crv, in0=g2v[:, sl_u[0]:sl_u[1], sl_v[0]:sl_v[1]],
                    scalar=ww[:, wcol:wcol + 1], in1=crv,
                    op0=ALU.mult, op1=ALU.add)
            p0 = sbg.tile([MP, N169 + 1], F32, tag="p0")
            nc.scalar.activation(p0[:, 0:N169], corr[:], Exp,
                                 accum_out=p0[:, N169:N169 + 1])
            z0_g[g] = p0
            bb = sbg.tile([MP, 196], BF16, tag="bb")
            nc.vector.memset(bb[:], 0.0)
            bbv = bb[:].rearrange("p (u v) -> p u v", u=14)
            p0v = p0[:, 0:N169].rearrange("p (i j) -> p i j", i=P13)
            nc.vector.tensor_scalar(bbv[:, 0:13, 0:13], p0v, ww[:, 0:1], None,
                                    op0=ALU.mult)
            for (sl_u, sl_v, wcol) in (((0, 13), (1, 14), 1),
                                       ((1, 14), (0, 13), 2),
                                       ((1, 14), (1, 14), 3)):
                dstv = bbv[:, sl_u[0]:sl_u[1], sl_v[0]:sl_v[1]]
                nc.vector.scalar_tensor_tensor(
                    out=dstv, in0=p0v, scalar=ww[:, wcol:wcol + 1], in1=dstv,
                    op0=ALU.mult, op1=ALU.add)
            qtv = bass.AP(qt[:].tensor, qt[:].offset,
                          [qt[:].ap[0], [32, 14], [448, 14], [1, C]])  # (u,v,c)
            bbb = bbv.to_broadcast([MP, 14, 14, C])
            nc.vector.tensor_tensor(qtv, qtv, bbb, op=ALU.mult)
            o0 = sbg.tile([MP, C], F32, tag="o0")
            pr = bass.AP(qt[:].tensor, qt[:].offset,
                         [qt[:].ap[0], [1, C], [32, 14], [448, 14]])
            nc.vector.tensor_reduce(o0[:], pr, axis=AX.XY, op=ALU.add)
            out0_g[g] = o0
            for yr in (2 * g, 2 * g + 1):
                if yr < RY:
                    emit_row(yr)
            if g == 0 and "dbg_qt" in outs:
                nc.sync.dma_start(outs["dbg_qt"], qt[:])
                nc.sync.dma_start(outs["dbg_g2"], g2[:])
                nc.sync.dma_start(outs["dbg_idx2qr"], idx2qr[:])
                nc.sync.dma_start(outs["dbg_idx2cv"], idx2cv[:])
                nc.sync.dma_start(outs["dbg_o0"], o0[:])
                nc.sync.dma_start(outs["dbg_bb"], bb[:])
                nc.sync.dma_start(outs["dbg_offs"], offs[:])
                nc.sync.dma_start(outs["dbg_e1"], e1[:])
                nc.sync.dma_start(outs["dbg_g1"], g1[:])




# ---------------- numpy mirror of one core (debug) ----------------
def core_reference(m):
    m = {k: (np.asarray(v, np.float32) if v.dtype != np.int32 else v)
         for k, v in m.items()}
    f1 = m["f1"].reshape(CF, RY, W)
    f2p0 = m["f2p0"].reshape(CF, H_SLAB, WCV)
    o0full = np.zeros((RY, W, C), np.float32)
    z0full = np.zeros((RY, W, 1), np.float32)
    for g, yg, MP in GROUPS:
        nrow = NROWS_G
        lhs = m["f1pair"][:, g * PPG:(g + 1) * PPG].astype(np.float32)
        cv = np.einsum('cp,crw->prw', lhs, f2p0[:, 2 * g:2 * g + nrow, :])
        cvf = np.ascontiguousarray(cv).reshape(-1)
        yy = (np.arange(MP) >= 64).astype(np.int64)
        rows = (R + yy[:, None, None] + 3 * np.arange(P13)[None, :, None])
        cvcc = np.take_along_axis(
            cv, np.broadcast_to(rows, (MP, P13, WCV)), axis=1)
        ccf = np.ascontiguousarray(cvcc).reshape(-1)
        sidx = m["idx1"][:MP, 0] - (np.arange(MP) * NROWS_G - np.arange(MP) * P13) * WCV
        # device sidx indexes cvcc directly: (p*13)*WCV + x + 6
        sidx = (np.arange(MP) * P13 * WCV
                + np.minimum(np.arange(MP) - 64 * yy, W - 1) + R)
        NS1 = 12 * WCV + CC_RUN
        g1 = ccf[sidx[:, None] + np.arange(NS1)[None, :]]
        cc0 = np.stack([g1[:, i * WCV + 3 * np.arange(P13)]
                        for i in range(P13)], 1).reshape(MP, N169)
        e1 = np.exp(cc0)
        S = e1.sum(1, keepdims=True)
        offx = np.clip((e1 * m["gridx"][:MP]).sum(1, keepdims=True) / S,
                       -MAXOFF, MAXOFF)
        offy = np.clip((e1 * m["gridy"][:MP]).sum(1, keepdims=True) / S,
                       -MAXOFF, MAXOFF)
        fbx = np.floor(offx + FLOOR_BIAS)
        fby = np.floor(offy + FLOOR_BIAS)
        wx = (offx + FLOOR_BIAS) - fbx
        wy = (offy + FLOOR_BIAS) - fby
        s2 = (fby.astype(np.int64) * WCV + fbx.astype(np.int64))
        idx2cv = (m["c2cv"][:MP, 0] + s2[:, 0]).astype(np.int64)
        idx2qr = (m["c2qr"][:MP, 0] + (s2[:, 0] + yg * WCV) * 448).astype(np.int64)
        NS2 = 13 * WCV + 14
        g2s = cvf[idx2cv[:, None] + np.arange(NS2)[None, :]]
        g2 = np.stack([g2s[:, u * WCV:u * WCV + 14] for u in range(14)], 1)
        qrf = m["qr0pad"].reshape(-1)
        qts = qrf[idx2qr[:, None] + np.arange(6272)[None, :]]
        # stream order (v, u, c) -> [MP, u, v, c]
        qt = qts.reshape(MP, 14, 14, C).transpose(0, 2, 1, 3)
        w00 = (1 - wy) * (1 - wx); w01 = (1 - wy) * wx
        w10 = wy * (1 - wx); w11 = wy * wx
        corr = (w00 * g2[:, 0:13, 0:13].reshape(MP, N169)
                + w01 * np.ascontiguousarray(g2[:, 0:13, 1:14]).reshape(MP, N169)
                + w10 * np.ascontiguousarray(g2[:, 1:14, 0:13]).reshape(MP, N169)
                + w11 * np.ascontiguousarray(g2[:, 1:14, 1:14]).reshape(MP, N169))
        p0 = np.exp(corr)
        z0 = p0.sum(1, keepdims=True)
        bb = np.zeros((MP, 14, 14), np.float32)
        p0v = p0.reshape(MP, P13, P13)
        bb[:, 0:13, 0:13] += w00[..., None] * p0v
        bb[:, 0:13, 1:14] += w01[..., None] * p0v
        bb[:, 1:14, 0:13] += w10[..., None] * p0v
        bb[:, 1:14, 1:14] += w11[..., None] * p0v
        o0 = (qt * bb[..., None]).sum((1, 2))
        for yloc in range(2):
            if yg + yloc >= RY:
                continue
            o0full[yg + yloc] = o0[64 * yloc:64 * yloc + W]
            z0full[yg + yloc] = z0[64 * yloc:64 * yloc + W]
    out = np.zeros((RY, W, C), np.float32)
    maskT = m["maskT"].reshape(WB, P13, W)[:, 0, :]
    for yr in range(RY):
        acc = np.zeros((W, C + 1), np.float32)
        for r in range(2):
            f2p = m[f"f2p{r + 1}"].reshape(CF, NRQ, WB)
            qrT = m[f"qrT{r + 1}"].reshape(WB, NRQ, C + 1)
            for iy in range(P13):
                ct = np.einsum('cq,cx->qx', f2p[:, yr + iy, :], f1[:, yr, :])
                em = np.exp(ct) * maskT
                acc += np.einsum('qx,qd->xd', em, qrT[:, yr + iy, :])
        den = acc[:, C:C + 1] + z0full[yr]
        out[yr] = (acc[:, :C] + o0full[yr]) / den
    return out


def full_reference_from_cores(in_maps):
    outs = [core_reference(in_maps[i]) for i in range(NCORES)]
    full = np.stack(outs, 0)            # [8, 7, 56, C]
    return full.reshape(H, W, C).transpose(2, 0, 1)[None]


DEBUG_SPECS = dict(
    dbg_qt=([128, 14 * 448], F32), dbg_g2=([128, 196], F32),
    dbg_idx2qr=([128, 14], I32), dbg_idx2cv=([128, 14], I32),
    dbg_o0=([128, C], F32), dbg_bb=([128, 196], F32),
    dbg_offs=([128, 2], F32), dbg_e1=([128, N169 + 1], F32),
    dbg_g1=([128, P13 * CC_RUN], F32),
)


def build_program(ncores=NCORES, debug=False):
    import concourse.bacc as bacc
    nc = bacc.Bacc("TRN2", target_bir_lowering=False, debug=False,
                   enable_asserts=True, num_devices=ncores)
    ins = {}
    for name, (shape, dt_) in INPUT_SPECS.items():
        ins[name] = nc.dram_tensor(name, shape, dt_, kind="ExternalInput").ap()
    outs = {"out": nc.dram_tensor("out", OUT_SPEC[0], OUT_SPEC[1],
                                  kind="ExternalOutput").ap()}
    if debug:
        for name, (shape, dt_) in DEBUG_SPECS.items():
            outs[name] = nc.dram_tensor(name, shape, dt_,
                                        kind="ExternalOutput").ap()
    with tile.TileContext(nc) as tc:
        build_kernel(tc, outs, ins)
    nc.compile()
    return nc


# ======================= runner =======================
import os as _os


def _build_program():
    import concourse.bacc as bacc
    nc = bacc.Bacc("TRN2", target_bir_lowering=False, debug=False,
                   enable_asserts=True, num_devices=NCORES)
    ins = {}
    for name, (shape, dt_) in INPUT_SPECS.items():
        ins[name] = nc.dram_tensor(name, shape, dt_, kind="ExternalInput").ap()
    outs = {"out": nc.dram_tensor("out", OUT_SPEC[0], OUT_SPEC[1],
                                  kind="ExternalOutput").ap()}
    with tile.TileContext(nc) as tc:
        build_kernel(tc, outs, ins)
    nc.compile()
    return nc


_LAST_RESULT = {}


def kernel(**inputs):
    from concourse.bass_utils import run_bass_kernel_spmd
    from concourse.bass_interp import get_hw_module

    in_maps = host_prep(**inputs)
    nc = _build_program()
    nc.m = get_hw_module(nc.m)
    trace = _os.environ.get("KERNEL_TRACE", "0") == "1"
    res = run_bass_kernel_spmd(
        nc, in_maps, core_ids=list(range(NCORES)), trace=trace)
    _LAST_RESULT["res"] = res
    slabs = [np.asarray(res.results[i]["out"], np.float32).reshape(RY, W, C)
             for i in range(NCORES)]
    full = np.concatenate(slabs, 0)          # [56, 56, 32]
    return np.ascontiguousarray(full.transpose(2, 0, 1)[None])

